# revision 9
# baseline (speedup 1.0000x reference)
"""BayesianLinear Trainium2 kernel, 8-core SPMD (data-parallel over batch).

Per-core computation (4 samples each):
    w_b = weight_mean + noise_b * exp(0.5 * weight_logvar)   (B,O,I)
    out_b = x_b @ w_b^T + bias                               (B,L,O)

Design (per core), v3 — bf16 matmul datapath, dual-ring DMA:
  - std = exp(.5 lv) once on ACT (cast to bf16 on write), O on partitions.
  - mean^T resident in bf16 (PE-transposed once at startup, cast on evac).
  - x/mean/logvar declared float32r (same bits as f32) so their PE
    transposes run 1.5 cyc/row instead of 2.0.
  - x loaded row-block-major ("(p m) i"): 16KB contiguous per partition
    descriptor; the induced L-permutation is undone for free by the
    partition->DRAM-row map of the output store.
  - Loads split across both HWDGE rings (a single ring saturates at
    ~306 B/ns): noise + x + bias on the SP ring, weights on the ACT ring;
    stores on the ACT ring.
  - Per sample, software-pipelined in column halves so PE stays dense:
      noise chunk: load f32 -> DVE/GpSimd scale-mul (bf16 out) ->
      PE transpose (bf16, 1 cyc/row) -> DVE fused mean-add evac (bf16 wT)
      matmuls: psum += xT.T @ wT over 8 k-tiles (bf16, 1 cyc/row),
      K=1 bias matmul, ACT evac (f32) into a full-width out slab,
      single 512KB store per (sample, row-block) with 4KB-contiguous rows.
  - Dummy keep-warm transposes woven into the DMA-bound prologue so the
    HAM clock gate never re-throttles the PE array.
"""
import numpy as np

SAMPLES = 4           # batch samples per core
N_CORES = 8
B, L, I, O = 32, 512, 1024, 1024
KT = I // 128         # 8 k-tiles (contraction)
OT = O // 128         # 8 o-blocks
LT = L // 128         # 4 l-tiles (row blocks)
NCH = 4               # noise chunks per sample (2 o-blocks each)

_cache = {}


def _split_multi_waits(nc, mybir):
    """This walrus build allows at most one sync-wait per instruction; move
    extra waits onto preceding single-wait NOPs on the same engine.  Safe
    because kernel semaphores are monotonic between resets, so waiting
    sequentially is equivalent to waiting on the conjunction."""
    for fn in nc.m.functions:
        for bb in fn.blocks:
            insts = bb.instructions
            changed = False
            new_list = []
            for inst in insts:
                si = inst.sync_info
                if si is not None and si.on_wait and len(si.on_wait) > 1:
                    waits = list(si.on_wait)
                    for j, w in enumerate(waits[:-1]):
                        nop = mybir.InstNoOp(name=f"{inst.name}-w{j}", ins=[], outs=[])
                        nop.engine = inst.engine
                        nop.sync_info = mybir.SyncInfo(on_wait=[w], on_update=[])
                        new_list.append(nop)
                    inst.sync_info = mybir.SyncInfo(
                        on_wait=[waits[-1]], on_update=list(si.on_update or []))
                    changed = True
                new_list.append(inst)
            if changed:
                bb.instructions = new_list


def build_nc(use_f32r=True):
    from contextlib import ExitStack
    from concourse import bass, mybir, tile, masks

    F32 = mybir.dt.float32
    F32R = mybir.dt.float32r
    BF16 = mybir.dt.bfloat16
    Exp = mybir.ActivationFunctionType.Exp
    Copy = mybir.ActivationFunctionType.Copy
    mult = mybir.AluOpType.mult
    add = mybir.AluOpType.add

    nc = bass.Bass()
    x_d = nc.declare_dram_parameter("x", [SAMPLES, L, I], F32, isOutput=False)
    nz_d = nc.declare_dram_parameter("noise", [SAMPLES, O, I], F32, isOutput=False)
    wm_d = nc.declare_dram_parameter("weight_mean", [O, I], F32, isOutput=False)
    wl_d = nc.declare_dram_parameter("weight_logvar", [O, I], F32, isOutput=False)
    b_d = nc.declare_dram_parameter("bias", [O], F32, isOutput=False)
    out_d = nc.declare_dram_parameter("out", [SAMPLES, L, O], F32, isOutput=True)

    with tile.TileContext(nc) as tc, ExitStack() as ctx:
        resident = ctx.enter_context(tc.tile_pool(name="resident", bufs=1))
        nz_pool = ctx.enter_context(tc.tile_pool(name="nz", bufs=4))
        sc_pool = ctx.enter_context(tc.tile_pool(name="scn", bufs=2))
        xnat_pool = ctx.enter_context(tc.tile_pool(name="xnat", bufs=2))
        xT_pool = ctx.enter_context(tc.tile_pool(name="xT", bufs=2))
        wT_pool = ctx.enter_context(tc.tile_pool(name="wT", bufs=2))
        out_pool = ctx.enter_context(tc.tile_pool(name="outp", bufs=2))
        psum_mm = ctx.enter_context(tc.tile_pool(name="psum_mm", bufs=2, space="PSUM"))
        psum_t = ctx.enter_context(tc.tile_pool(name="psum_t", bufs=3, space="PSUM"))
        psum_w = ctx.enter_context(tc.tile_pool(name="psum_w", bufs=1, space="PSUM"))

        # ---------------- one-time setup ----------------
        std_nat = resident.tile([128, OT, I], BF16, tag="std")   # exp(.5 lv), natural
        meanT = resident.tile([128, KT, O], BF16, tag="meanT")   # mean^T
        ident_f = resident.tile([128, 128], F32, tag="ident_f")
        ident_r = resident.tile([128, 128], F32R, tag="ident_r")
        ident_b = resident.tile([128, 128], BF16, tag="ident_b")
        ones_b = resident.tile([1, 128], BF16, tag="ones_b")
        bias_f = resident.tile([1, O], F32, tag="bias_f")
        bias_b = resident.tile([1, O], BF16, tag="bias_b")

        masks.make_identity(nc, ident_f[:])
        nc.vector.tensor_copy(ident_r[:], ident_f[:])
        nc.vector.tensor_copy(ident_b[:], ident_f[:])
        nc.gpsimd.memset(ones_b[:], 1.0)
        nc.sync.dma_start(bias_f[:], b_d[:].rearrange("(a o) -> a o", a=1))
        nc.vector.tensor_copy(bias_b[:], bias_f[:])

        # PE pre-warm + keep-warm: dummy transposes (result never read) keep
        # the HAM activity window busy so the 2.4 GHz clock stays ungated
        # through the DMA-bound prologue.
        warm = resident.tile([128, 128], F32, tag="warm")
        nc.gpsimd.memset(warm[:], 0.001)
        pwarm = psum_w.tile([128, 128], F32, tag="pw")

        def emit_warm(n):
            for _ in range(n):
                nc.tensor.matmul(pwarm[:], warm[:], warm[:],
                                 is_transpose=True, start=True, stop=True)

        emit_warm(16)

        def emit_mean_slab(j):
            """load + exp + transpose weight slab j (o-blocks 2j, 2j+1)."""
            sl = slice(256 * j, 256 * (j + 1))
            mt = nz_pool.tile([128, 2, I], F32R, tag="nz", name=f"mt{j}")
            lt = nz_pool.tile([128, 2, I], F32, tag="nz", name=f"lt{j}")
            nc.scalar.dma_start(
                mt[:], wm_d[sl, :].rearrange("(q p) i -> p q i", p=128).bitcast(F32R))
            nc.scalar.dma_start(
                lt[:], wl_d[sl, :].rearrange("(q p) i -> p q i", p=128))
            nc.scalar.activation(std_nat[:, 2 * j:2 * (j + 1), :], lt[:],
                                 Exp, bias=0.0, scale=0.5)
            for q in range(2):
                ob = 2 * j + q
                for kh in range(2):  # k halves of 4
                    px = psum_t.tile([128, 4, 128], F32R, tag="pt")
                    for kk in range(4):
                        k = 4 * kh + kk
                        nc.tensor.matmul(
                            px[:, kk, :], mt[:, q, 128 * k:128 * (k + 1)], ident_r[:],
                            is_transpose=True, start=True, stop=True)
                    dst = meanT[:, 4 * kh:4 * (kh + 1), 128 * ob:128 * (ob + 1)]
                    if q == 0:
                        nc.scalar.activation(dst, px[:], Copy)
                    else:
                        nc.vector.tensor_copy(dst, px[:])

        # ---------------- per-sample pipeline ----------------
        def emit_chunk(b, c, wT):
            """noise chunk c (o-blocks 2c, 2c+1): load, scale (bf16),
            transpose, fused mean-add into wT."""
            nz = nz_pool.tile([128, 2, I], F32, tag="nz")
            nc.sync.dma_start(
                nz[:], nz_d[b, 256 * c:256 * (c + 1), :].rearrange("(q p) i -> p q i", p=128))
            sc = sc_pool.tile([128, 2, I], BF16, tag="scn")
            eng = nc.gpsimd if c in (1, 3) else nc.vector
            eng.tensor_tensor(sc[:], nz[:], std_nat[:, 2 * c:2 * (c + 1), :], mult)
            for q in range(2):
                ob = 2 * c + q
                for kh in range(2):
                    pn = psum_t.tile([128, 4, 128], BF16, tag="pt")
                    for kk in range(4):
                        k = 4 * kh + kk
                        nc.tensor.matmul(
                            pn[:, kk, :], sc[:, q, 128 * k:128 * (k + 1)],
                            ident_b[:], is_transpose=True, start=True, stop=True)
                    nc.vector.tensor_tensor(
                        wT[:, 4 * kh:4 * (kh + 1), 128 * ob:128 * (ob + 1)],
                        pn[:], meanT[:, 4 * kh:4 * (kh + 1), 128 * ob:128 * (ob + 1)],
                        add)

        def emit_xT(xT, x_nat):
            """PE-transpose x (f32r, 1.5 cyc/row) with cast-to-bf16 on ACT evac."""
            for m in range(LT):
                for kh in range(2):
                    px = psum_t.tile([128, 4, 128], F32R, tag="pt")
                    for kk in range(4):
                        k = 4 * kh + kk
                        nc.tensor.matmul(
                            px[:, kk, :], x_nat[:, m, 128 * k:128 * (k + 1)], ident_r[:],
                            is_transpose=True, start=True, stop=True)
                    nc.scalar.activation(
                        xT[:, 4 * kh:4 * (kh + 1), 128 * m:128 * (m + 1)], px[:], Copy)

        def emit_mm_half(b, n, wT, xT, osb):
            """matmuls for output columns [512n, 512(n+1)); store full rows
            after the second half."""
            for m in range(LT):
                pm = psum_mm.tile([128, 512], F32, tag="pmm")
                for k in range(KT):
                    nc.tensor.matmul(pm[:], xT[:, k, 128 * m:128 * (m + 1)],
                                     wT[:, k, 512 * n:512 * (n + 1)],
                                     start=(k == 0), stop=False)
                nc.tensor.matmul(pm[:], ones_b[:], bias_b[:, 512 * n:512 * (n + 1)],
                                 start=False, stop=True)
                nc.scalar.activation(osb[:, m, 512 * n:512 * (n + 1)], pm[:], Copy)
                if n == 1:
                    nc.scalar.dma_start(
                        out_d[b].rearrange("(p m) o -> p m o", m=LT)[:, m, :],
                        osb[:, m, :])

        x_tiles = {0: xnat_pool.tile([128, LT, I], F32R, tag="xnat", name="xn0")}
        nc.sync.dma_start(x_tiles[0][:],
                          x_d[0].rearrange("(p m) i -> p m i", m=LT).bitcast(F32R))
        for b in range(SAMPLES):
            xT = xT_pool.tile([128, KT, L], BF16, tag="xT")
            emit_xT(xT, x_tiles.pop(b))
            wT = wT_pool.tile([128, KT, O], BF16, tag="wT")
            osb = out_pool.tile([128, LT, O], F32, tag="out")
            for half in range(2):
                for cc in (2 * half, 2 * half + 1):
                    if b == 0:
                        emit_mean_slab(cc)
                        emit_warm(6)
                    emit_chunk(b, cc, wT)
                    if b == 0:
                        emit_warm(6)
                    if b + 1 < SAMPLES and half == 0 and cc == 1:
                        xn_next = xnat_pool.tile([128, LT, I], F32R, tag="xnat",
                                                 name=f"xn{b+1}")
                        x_tiles[b + 1] = xn_next
                        nc.sync.dma_start(
                            xn_next[:],
                            x_d[b + 1].rearrange("(p m) i -> p m i", m=LT).bitcast(F32R))
                emit_mm_half(b, half, wT, xT, osb)

    _split_multi_waits(nc, mybir)
    return nc


def _get_nc(use_f32r=True):
    key = ("nc", use_f32r)
    if key not in _cache:
        _cache[key] = build_nc(use_f32r)
    return _cache[key]


def kernel(x, weight_mean, weight_logvar, bias, noise):
    from concourse import bass_utils

    x = np.ascontiguousarray(x, dtype=np.float32)
    noise = np.ascontiguousarray(noise, dtype=np.float32)
    weight_mean = np.ascontiguousarray(weight_mean, dtype=np.float32)
    weight_logvar = np.ascontiguousarray(weight_logvar, dtype=np.float32)
    bias = np.ascontiguousarray(bias, dtype=np.float32)

    nc = _get_nc()
    in_maps = []
    for c in range(N_CORES):
        sl = slice(SAMPLES * c, SAMPLES * (c + 1))
        in_maps.append({
            "x": x[sl], "noise": noise[sl],
            "weight_mean": weight_mean, "weight_logvar": weight_logvar,
            "bias": bias,
        })
    res = bass_utils.run_bass_kernel_spmd(nc, in_maps, list(range(N_CORES)))
    out = np.concatenate([res.results[c]["out"] for c in range(N_CORES)], axis=0)
    return out.astype(np.float32)


# revision 12
# speedup vs baseline: 1.1148x; 1.1148x over previous
"""BayesianLinear Trainium2 kernel, 8-core SPMD (data-parallel over batch).

Per-core computation (4 samples each):
    w_b = weight_mean + noise_b * exp(0.5 * weight_logvar)   (B,O,I)
    out_b = x_b @ w_b^T + bias                               (B,L,O)

Design (per core), v4 — bf16 datapath, 1024-wide matmuls, chunk interleave:
  - All loads on the SP HWDGE ring in strict need order (the HBM system
    caps ~310 B/ns aggregate; ordering, not ring count, is what matters).
    Stores on the ACT ring.
  - std = exp(.5 lv) once on ACT (bf16 out); mean^T resident bf16.
  - x/mean bitcast to f32r for 1.5 cyc/row PE transposes.
  - x loaded row-block-major ("(p m) i"): 16KB contiguous per-partition
    descriptors; the induced L-permutation is undone for free by the
    partition->DRAM-row map of the output store.
  - Full-width wT (bf16) per sample; matmuls stream 1024 columns per
    instruction into a 2-bank PSUM tile (one LDWEIGHTS per k-tile).
  - Noise chunks of sample b+1 are interleaved between the matmul
    row-blocks of sample b, so PE alternates mm bursts and transpose
    bursts with no long starvation, and the tail after the last noise
    load is just one sample's matmuls.
  - Dummy keep-warm transposes woven into the DMA-bound prologue so the
    HAM clock gate never re-throttles the PE array.
"""
import numpy as np

SAMPLES = 4           # batch samples per core
N_CORES = 8
B, L, I, O = 32, 512, 1024, 1024
KT = I // 128         # 8 k-tiles (contraction)
OT = O // 128         # 8 o-blocks
LT = L // 128         # 4 l-tiles (row blocks)
NCH = 4               # noise chunks per sample (2 o-blocks each)

_cache = {}


def _split_multi_waits(nc, mybir):
    """This walrus build allows at most one sync-wait per instruction; move
    extra waits onto preceding single-wait NOPs on the same engine.  Safe
    because kernel semaphores are monotonic between resets, so waiting
    sequentially is equivalent to waiting on the conjunction."""
    for fn in nc.m.functions:
        for bb in fn.blocks:
            insts = bb.instructions
            changed = False
            new_list = []
            for inst in insts:
                si = inst.sync_info
                if si is not None and si.on_wait and len(si.on_wait) > 1:
                    waits = list(si.on_wait)
                    for j, w in enumerate(waits[:-1]):
                        nop = mybir.InstNoOp(name=f"{inst.name}-w{j}", ins=[], outs=[])
                        nop.engine = inst.engine
                        nop.sync_info = mybir.SyncInfo(on_wait=[w], on_update=[])
                        new_list.append(nop)
                    inst.sync_info = mybir.SyncInfo(
                        on_wait=[waits[-1]], on_update=list(si.on_update or []))
                    changed = True
                new_list.append(inst)
            if changed:
                bb.instructions = new_list


def build_nc(use_f32r=True):
    from contextlib import ExitStack
    from concourse import bass, mybir, tile, masks

    F32 = mybir.dt.float32
    F32R = mybir.dt.float32r
    BF16 = mybir.dt.bfloat16
    Exp = mybir.ActivationFunctionType.Exp
    Copy = mybir.ActivationFunctionType.Copy
    mult = mybir.AluOpType.mult
    add = mybir.AluOpType.add

    nc = bass.Bass()
    x_d = nc.declare_dram_parameter("x", [SAMPLES, L, I], F32, isOutput=False)
    nz_d = nc.declare_dram_parameter("noise", [SAMPLES, O, I], F32, isOutput=False)
    wm_d = nc.declare_dram_parameter("weight_mean", [O, I], F32, isOutput=False)
    wl_d = nc.declare_dram_parameter("weight_logvar", [O, I], F32, isOutput=False)
    b_d = nc.declare_dram_parameter("bias", [O], F32, isOutput=False)
    out_d = nc.declare_dram_parameter("out", [SAMPLES, L, O], F32, isOutput=True)

    with tile.TileContext(nc) as tc, ExitStack() as ctx:
        resident = ctx.enter_context(tc.tile_pool(name="resident", bufs=1))
        nz_pool = ctx.enter_context(tc.tile_pool(name="nz", bufs=4))
        sc_pool = ctx.enter_context(tc.tile_pool(name="scn", bufs=2))
        xnat_pool = ctx.enter_context(tc.tile_pool(name="xnat", bufs=2))
        xT_pool = ctx.enter_context(tc.tile_pool(name="xT", bufs=2))
        wT_pool = ctx.enter_context(tc.tile_pool(name="wT", bufs=2))
        out_pool = ctx.enter_context(tc.tile_pool(name="outp", bufs=2))
        psum_mm = ctx.enter_context(tc.tile_pool(name="psum_mm", bufs=4, space="PSUM"))
        psum_t = ctx.enter_context(tc.tile_pool(name="psum_t", bufs=3, space="PSUM"))
        psum_w = ctx.enter_context(tc.tile_pool(name="psum_w", bufs=1, space="PSUM"))

        # ---------------- one-time setup ----------------
        std_nat = resident.tile([128, OT, I], BF16, tag="std")   # exp(.5 lv), natural
        meanT = resident.tile([128, KT, O], BF16, tag="meanT")   # mean^T
        ident_f = resident.tile([128, 128], F32, tag="ident_f")
        ident_r = resident.tile([128, 128], F32R, tag="ident_r")
        ident_b = resident.tile([128, 128], BF16, tag="ident_b")
        ones_b = resident.tile([1, 128], BF16, tag="ones_b")
        bias_f = resident.tile([1, O], F32, tag="bias_f")
        bias_b = resident.tile([1, O], BF16, tag="bias_b")

        masks.make_identity(nc, ident_f[:])
        nc.vector.tensor_copy(ident_r[:], ident_f[:])
        nc.vector.tensor_copy(ident_b[:], ident_f[:])
        nc.gpsimd.memset(ones_b[:], 1.0)
        nc.sync.dma_start(bias_f[:], b_d[:].rearrange("(a o) -> a o", a=1))
        nc.vector.tensor_copy(bias_b[:], bias_f[:])

        # PE pre-warm + keep-warm: dummy transposes (result never read) keep
        # the HAM activity window busy so the 2.4 GHz clock stays ungated
        # through the DMA-bound prologue.
        warm = resident.tile([128, 128], F32, tag="warm")
        nc.gpsimd.memset(warm[:], 0.001)
        pwarm = psum_w.tile([128, 128], F32, tag="pw")

        def emit_warm(n):
            for _ in range(n):
                nc.tensor.matmul(pwarm[:], warm[:], warm[:],
                                 is_transpose=True, start=True, stop=True)

        emit_warm(16)

        def emit_mean_slab(j):
            """load + exp + transpose weight slab j (o-blocks 2j, 2j+1)."""
            sl = slice(256 * j, 256 * (j + 1))
            mt = nz_pool.tile([128, 2, I], F32R, tag="nz", name=f"mt{j}")
            lt = nz_pool.tile([128, 2, I], F32, tag="nz", name=f"lt{j}")
            nc.sync.dma_start(
                mt[:], wm_d[sl, :].rearrange("(q p) i -> p q i", p=128).bitcast(F32R))
            nc.sync.dma_start(
                lt[:], wl_d[sl, :].rearrange("(q p) i -> p q i", p=128))
            nc.scalar.activation(std_nat[:, 2 * j:2 * (j + 1), :], lt[:],
                                 Exp, bias=0.0, scale=0.5)
            for q in range(2):
                ob = 2 * j + q
                for kh in range(2):  # k halves of 4
                    px = psum_t.tile([128, 4, 128], F32R, tag="pt")
                    for kk in range(4):
                        k = 4 * kh + kk
                        nc.tensor.matmul(
                            px[:, kk, :], mt[:, q, 128 * k:128 * (k + 1)], ident_r[:],
                            is_transpose=True, start=True, stop=True)
                    dst = meanT[:, 4 * kh:4 * (kh + 1), 128 * ob:128 * (ob + 1)]
                    if q == 0:
                        nc.scalar.activation(dst, px[:], Copy)
                    else:
                        nc.vector.tensor_copy(dst, px[:])

        # ---------------- per-sample pipeline ----------------
        def emit_chunk(b, c, wT):
            """noise chunk c (o-blocks 2c, 2c+1): load, scale (bf16),
            transpose, fused mean-add into wT."""
            nz = nz_pool.tile([128, 2, I], F32, tag="nz")
            nc.sync.dma_start(
                nz[:], nz_d[b, 256 * c:256 * (c + 1), :].rearrange("(q p) i -> p q i", p=128))
            sc = sc_pool.tile([128, 2, I], BF16, tag="scn")
            eng = nc.gpsimd if c in (1, 3) else nc.vector
            eng.tensor_tensor(sc[:], nz[:], std_nat[:, 2 * c:2 * (c + 1), :], mult)
            for q in range(2):
                ob = 2 * c + q
                for kh in range(2):
                    pn = psum_t.tile([128, 4, 128], BF16, tag="pt")
                    for kk in range(4):
                        k = 4 * kh + kk
                        nc.tensor.matmul(
                            pn[:, kk, :], sc[:, q, 128 * k:128 * (k + 1)],
                            ident_b[:], is_transpose=True, start=True, stop=True)
                    nc.vector.tensor_tensor(
                        wT[:, 4 * kh:4 * (kh + 1), 128 * ob:128 * (ob + 1)],
                        pn[:], meanT[:, 4 * kh:4 * (kh + 1), 128 * ob:128 * (ob + 1)],
                        add)

        def emit_xT(xT, x_nat):
            """PE-transpose x (f32r, 1.5 cyc/row) with cast-to-bf16 on ACT evac."""
            for m in range(LT):
                for kh in range(2):
                    px = psum_t.tile([128, 4, 128], F32R, tag="pt")
                    for kk in range(4):
                        k = 4 * kh + kk
                        nc.tensor.matmul(
                            px[:, kk, :], x_nat[:, m, 128 * k:128 * (k + 1)], ident_r[:],
                            is_transpose=True, start=True, stop=True)
                    nc.scalar.activation(
                        xT[:, 4 * kh:4 * (kh + 1), 128 * m:128 * (m + 1)], px[:], Copy)

        def emit_mm_row(b, m, wT, xT, osb):
            """full-width matmuls for row-block m: 8 k-tiles x 2 column
            halves (shared stationary per k), K=1 bias matmuls, evac, store."""
            pm0 = psum_mm.tile([128, 512], F32, tag="pmm")
            pm1 = psum_mm.tile([128, 512], F32, tag="pmm")
            for k in range(KT):
                xk = xT[:, k, 128 * m:128 * (m + 1)]
                nc.tensor.matmul(pm0[:], xk, wT[:, k, 0:512],
                                 start=(k == 0), stop=False)
                nc.tensor.matmul(pm1[:], xk, wT[:, k, 512:1024],
                                 start=(k == 0), stop=False)
            nc.tensor.matmul(pm0[:], ones_b[:], bias_b[:, 0:512],
                             start=False, stop=True)
            nc.tensor.matmul(pm1[:], ones_b[:], bias_b[:, 512:1024],
                             start=False, stop=True)
            nc.scalar.activation(osb[:, m, 0:512], pm0[:], Copy)
            nc.scalar.activation(osb[:, m, 512:1024], pm1[:], Copy)
            nc.scalar.dma_start(
                out_d[b].rearrange("(p m) o -> p m o", m=LT)[:, m, :],
                osb[:, m, :])

        def load_x(b):
            xn = xnat_pool.tile([128, LT, I], F32R, tag="xnat", name=f"xn{b}")
            nc.sync.dma_start(
                xn[:], x_d[b].rearrange("(p m) i -> p m i", m=LT).bitcast(F32R))
            return xn

        # prologue: sample 0's weights + noise, interleaved with keep-warm
        x_tiles = {0: load_x(0)}
        xTs = {0: xT_pool.tile([128, KT, L], BF16, tag="xT", name="xT0")}
        emit_xT(xTs[0], x_tiles.pop(0))
        wTs = {0: wT_pool.tile([128, KT, O], BF16, tag="wT", name="wT0")}
        for j in range(NCH):
            emit_mean_slab(j)
            emit_warm(4)
            emit_chunk(0, j, wTs[0])
            emit_warm(4)
        x_tiles[1] = load_x(1)

        for b in range(SAMPLES):
            osb = out_pool.tile([128, LT, O], F32, tag="out")
            if b + 1 < SAMPLES:
                wTs[b + 1] = wT_pool.tile([128, KT, O], BF16, tag="wT",
                                          name=f"wT{b+1}")
            for m in range(LT):
                emit_mm_row(b, m, wTs[b], xTs[b], osb)
                if b + 1 < SAMPLES:
                    emit_chunk(b + 1, m, wTs[b + 1])
            wTs.pop(b)
            if b + 1 < SAMPLES:
                xTs[b + 1] = xT_pool.tile([128, KT, L], BF16, tag="xT",
                                          name=f"xT{b+1}")
                emit_xT(xTs[b + 1], x_tiles.pop(b + 1))
                xTs.pop(b)
                if b + 2 < SAMPLES:
                    x_tiles[b + 2] = load_x(b + 2)

    _split_multi_waits(nc, mybir)
    return nc


def _get_nc(use_f32r=True):
    key = ("nc", use_f32r)
    if key not in _cache:
        _cache[key] = build_nc(use_f32r)
    return _cache[key]


def kernel(x, weight_mean, weight_logvar, bias, noise):
    from concourse import bass_utils

    x = np.ascontiguousarray(x, dtype=np.float32)
    noise = np.ascontiguousarray(noise, dtype=np.float32)
    weight_mean = np.ascontiguousarray(weight_mean, dtype=np.float32)
    weight_logvar = np.ascontiguousarray(weight_logvar, dtype=np.float32)
    bias = np.ascontiguousarray(bias, dtype=np.float32)

    nc = _get_nc()
    in_maps = []
    for c in range(N_CORES):
        sl = slice(SAMPLES * c, SAMPLES * (c + 1))
        in_maps.append({
            "x": x[sl], "noise": noise[sl],
            "weight_mean": weight_mean, "weight_logvar": weight_logvar,
            "bias": bias,
        })
    res = bass_utils.run_bass_kernel_spmd(nc, in_maps, list(range(N_CORES)))
    out = np.concatenate([res.results[c]["out"] for c in range(N_CORES)], axis=0)
    return out.astype(np.float32)
